# revision 20
# baseline (speedup 1.0000x reference)
"""Self-contained Trainium2 kernel for nn_AMDOptimizedAttention.

Reference computes, for B=2, S=2048, H=2048, nh=16, hd=128:
    q/k/v = hs @ w{q,k,v}.T  (torch Linear convention)
    q, k  = rope(q), rope(k)
    out   = causal_softmax(q @ k.T / sqrt(hd)) @ v
    y     = out @ wo.T

Sharding (Megatron-style tensor parallel over heads + data parallel over
batch): core c handles batch c//4, heads 4*(c%4) .. 4*(c%4)+3.  Each core
computes a partial y for its batch (row-sharded wo); host sums the 4
partials per batch (the "all-reduce" is done on host since kernel() must
return the full output anyway).

v2 layout: everything SBUF-resident per core, no DRAM spill.
  - x (hidden.T), v, exp(scores), wo in bf16; q/k (the softmax-sensitive
    chain) in float32r (fp22).  All matmuls accumulate fp32 in PSUM and
    run at 1 col/cycle.
  - scores computed transposed [k, q]; softmax sum over k via a
    ones-column matmul accumulated in PSUM next to the PV matmul; the
    1/sum row is broadcast across partitions with a K=1 matmul.
  - v-projection runs first, then k, then q, so attention on head h can
    start while later heads still project.
"""

import sys

if "/opt/trn_rl_repo" not in sys.path:
    sys.path.insert(0, "/opt/trn_rl_repo")

import numpy as np

B, S, H = 2, 2048, 2048
NH, HD = 16, 128
P = 128
NCORES = 8
HPC = 4              # heads per core
DSL = HPC * HD       # 512: per-core slice of the hidden dim
KO = H // P          # 16 contraction chunks for projections
TBP = 512            # projection token-block
QB = 512             # attention query-block
NQB = S // QB        # 4
SCALE = 1.0 / np.sqrt(HD)
ROPE_BASE = 10000.0
NEG = -1.0e30

_CACHE = {}


def _build_nc():
    import concourse.mybir as mybir
    from concourse import bacc
    from concourse.tile import TileContext

    f32 = mybir.dt.float32
    f32r = mybir.dt.float32r
    bf16 = mybir.dt.bfloat16
    Alu = mybir.AluOpType
    Act = mybir.ActivationFunctionType

    nc = bacc.Bacc("TRN2", target_bir_lowering=False)

    xT = nc.declare_dram_parameter("xT", [H, S], bf16, isOutput=False)
    wqT = nc.declare_dram_parameter("wqT", [H, DSL], bf16, isOutput=False)
    wkT = nc.declare_dram_parameter("wkT", [H, DSL], bf16, isOutput=False)
    wvT = nc.declare_dram_parameter("wvT", [H, DSL], bf16, isOutput=False)
    woT = nc.declare_dram_parameter("woT", [DSL, H], bf16, isOutput=False)
    # rope tables packed [128, S]: rows 0:64 cos, rows 64:128 sin
    csa = nc.declare_dram_parameter("csa", [P, S], f32, isOutput=False)
    csb = nc.declare_dram_parameter("csb", [P, S], f32, isOutput=False)
    maskp = nc.declare_dram_parameter("mask", [QB // P, P, QB], bf16, isOutput=False)
    onesb = nc.declare_dram_parameter("onesb", [P, P], bf16, isOutput=False)
    yout = nc.declare_dram_parameter("out", [S, H], f32, isOutput=True)

    xTr = xT.rearrange("(ko p) t -> p ko t", p=P)
    wT = {"q": wqT, "k": wkT, "v": wvT}
    wTr = {k: v.rearrange("(ko p) d -> p ko d", p=P) for k, v in wT.items()}

    def mm(ps, lhsT, rhs, start, stop):
        nc.tensor.matmul(ps, lhsT, rhs, start=start, stop=stop)

    with TileContext(nc) as tc, nc.allow_low_precision(
        reason="bf16/f32r staging is deliberate; matmuls accumulate in f32 PSUM"
    ):
        with (
            tc.tile_pool(name="res", bufs=1) as rpool,
            tc.tile_pool(name="xres", bufs=1) as xpool,
            tc.tile_pool(name="wvpool", bufs=1) as wvpool,
        ):
            # ---- residents ----
            xs = [xpool.tile([P, KO, S // 4], bf16, tag=f"xs{g}", name=f"xs{g}")
                  for g in range(4)]            # x.T, token-column chunks
            qT = [rpool.tile([P, S], bf16, tag=f"qT{h}", name=f"qT{h}")
                  for h in range(HPC)]
            kT = [rpool.tile([P, S], bf16, tag=f"kT{h}", name=f"kT{h}")
                  for h in range(HPC)]
            vs = rpool.tile([P, KO, DSL], bf16, tag="vs", name="vs")
            ao = [rpool.tile([P, S], bf16, tag=f"ao{h}", name=f"ao{h}")
                  for h in range(HPC)]
            wv = wvpool.tile([P, KO, DSL], bf16, tag="wv", name="wv")
            nc.sync.dma_start(wv[:], wTr["v"][:])
            TC = S // 4
            for g in range(4):
                nc.sync.dma_start(xs[g][:], xTr[:, :, g * TC:(g + 1) * TC])
            csA = rpool.tile([P, S], f32, tag="csA", name="csA")
            nc.sync.dma_start(csA[:], csa[:])
            csB = rpool.tile([P, S], f32, tag="csB", name="csB")
            nc.sync.dma_start(csB[:], csb[:])
            masks = rpool.tile([P, QB // P, QB], bf16, tag="masks", name="masks")
            nc.sync.dma_start(masks[:], maskp.rearrange("j p f -> p j f"))
            oneb = rpool.tile([P, P], bf16, tag="oneb", name="oneb")
            nc.sync.dma_start(oneb[:], onesb[:])
            wos = rpool.tile([P, DSL // P, H], bf16, tag="wos", name="wos")
            nc.sync.dma_start(wos[:], woT.rearrange("(ko p) e -> p ko e", p=P))

            def xsl(ko, t0, t1):
                g = t0 // TC
                assert t1 <= (g + 1) * TC
                return xs[g][:, ko, t0 - g * TC:t1 - g * TC]

            # ---- projections ----
            with (
                tc.tile_pool(name="wstream", bufs=2) as wpool,
                tc.tile_pool(name="ropetmp", bufs=2) as rtpool,
                tc.tile_pool(name="psA", bufs=1, space="PSUM") as psA,
            ):
                # k projections first (PE can start on 0.26MB of weights),
                # then v, then q
                NTB = S // TBP
                def v_projection():
                    for tt in range(KO):
                        pv = psA.tile([P, DSL], f32, tag="pv", name="pv", bufs=2)
                        for ko in range(KO):
                            mm(pv, xsl(ko, tt * P, (tt + 1) * P), wv[:, ko, :],
                               ko == 0, ko == KO - 1)
                        nc.vector.tensor_copy(vs[:, tt, :], pv[:])

                for pj, dstl in (("k", kT), ("q", qT)):
                    if pj == "q":
                        v_projection()
                    for h in range(HPC):
                        dsl = slice(h * P, (h + 1) * P)
                        wph = wpool.tile([P, KO, P], bf16, tag="wph", name="wph")
                        nc.sync.dma_start(wph[:], wTr[pj][:, :, dsl])
                        pss4 = {}
                        for pair in range(NTB // 2):
                            for i in range(2):
                                tb = pair * 2 + i
                                pss4[tb] = psA.tile([P, TBP], f32, tag=f"psqk{i}",
                                                    name=f"psqk{i}", bufs=2)
                            for ko in range(KO):
                                for tb in (pair * 2, pair * 2 + 1):
                                    mm(pss4[tb], wph[:, ko, :],
                                       xsl(ko, tb * TBP, (tb + 1) * TBP),
                                       ko == 0, ko == KO - 1)
                        for tb in range(NTB):
                            tslc = slice(tb * TBP, (tb + 1) * TBP)
                            ps = pss4[tb]
                            # m1 = [x1*cos; x2*cos] (SBUF), m2 = [x1*sin; x2*sin] (PSUM)
                            m1 = rtpool.tile([P, TBP], f32, tag="m1", name="m1")
                            m2 = psA.tile([P, TBP], f32, tag="m2", name="m2", bufs=2)
                            dst = dstl[h]
                            nc.vector.tensor_tensor(m1[:], ps[:], csA[:, tslc], Alu.mult)
                            nc.vector.tensor_tensor(m2[:], ps[:], csB[:, tslc], Alu.mult)
                            nc.vector.tensor_tensor(
                                dst[0:64, tslc], m1[0:64, :], m2[64:128, :], Alu.subtract)
                            nc.vector.tensor_tensor(
                                dst[64:128, tslc], m1[64:128, :], m2[0:64, :], Alu.add)

            # ---- attention (scores transposed [k, q]) + fused out-proj ----
            with (
                tc.tile_pool(name="et", bufs=4) as epool,
                tc.tile_pool(name="nrm", bufs=3) as npool,
                tc.tile_pool(name="pss", bufs=2, space="PSUM") as pss,
                tc.tile_pool(name="pso", bufs=2, space="PSUM") as pso,
                tc.tile_pool(name="psd", bufs=1, space="PSUM") as psd,
                tc.tile_pool(name="psy", bufs=2, space="PSUM") as psy,
            ):
                for qb in range(NQB):
                    qsl = slice(qb * QB, (qb + 1) * QB)
                    nkt = (qb + 1) * (QB // P)
                    for h in range(HPC):
                        po = pso.tile([P, QB], f32, tag="po", name="po")
                        pd = psd.tile([P, QB], f32, tag="pd", name="pd")
                        for kt in range(nkt):
                            pscr = pss.tile([P, QB], f32, tag="pscr", name="pscr")
                            mm(pscr, kT[h][:, kt * P:(kt + 1) * P],
                               qT[h][:, qsl], True, True)
                            j = kt - qb * (QB // P)
                            if j >= 0:
                                nc.vector.tensor_tensor(
                                    pscr[:], pscr[:], masks[:, j, :], Alu.add
                                )
                            et = epool.tile([P, QB], bf16, tag="et", name="et")
                            nc.scalar.activation(et[:], pscr[:], Act.Exp, scale=float(SCALE))
                            mm(po, vs[:, kt, h * P:(h + 1) * P], et[:],
                               kt == 0, kt == nkt - 1)
                            mm(pd, oneb[:], et[:], kt == 0, kt == nkt - 1)
                        rec = npool.tile([P, QB], f32, tag="rec", name="rec")
                        nc.vector.reciprocal(rec[:], pd[:])
                        nc.vector.tensor_tensor(
                            ao[h][:, qsl], po[:], rec[:], Alu.mult
                        )
                    # out-proj for this qb's token range, straight PSUM->DRAM
                    for tt in range(qb * (QB // P), (qb + 1) * (QB // P)):
                        tsl = slice(tt * P, (tt + 1) * P)
                        for ec in range(H // QB):
                            py = psy.tile([P, QB], f32, tag="py", name="py")
                            for dc in range(DSL // P):
                                mm(py, ao[dc][:, tsl],
                                   wos[:, dc, ec * QB:(ec + 1) * QB],
                                   dc == 0, dc == DSL // P - 1)
                            ysc = npool.tile([P, QB], f32, tag="ysc",
                                             name="ysc", bufs=3)
                            nc.vector.tensor_copy(ysc[:], py[:])
                            nc.sync.dma_start(
                                yout[tsl, ec * QB:(ec + 1) * QB], ysc[:])

    nc.finalize()
    return nc


def _host_inputs(hidden_states, wq, wk, wv, wo):
    import ml_dtypes

    f32 = np.float32
    bf = ml_dtypes.bfloat16
    ca = np.ascontiguousarray

    inv = 1.0 / (ROPE_BASE ** (np.arange(0, HD, 2, dtype=f32) / HD))
    t = np.arange(S, dtype=f32)
    fr = np.outer(t, inv)                      # [S, 64]
    cosT = np.cos(fr).T.astype(f32)            # [64, S]
    sinT = np.sin(fr).T.astype(f32)
    csa = ca(np.concatenate([cosT, cosT], axis=0))            # [128, S]
    csb = ca(np.concatenate([sinT, sinT], axis=0))

    jj, pp, ff = np.meshgrid(
        np.arange(QB // P), np.arange(P), np.arange(QB), indexing="ij"
    )
    mask = np.where(jj * P + pp > ff, f32(NEG), f32(0.0)).astype(bf)
    onesb = np.ones((P, P), bf)

    xTb = [ca(hidden_states[b].T.astype(bf)) for b in range(B)]

    in_maps = []
    for c in range(NCORES):
        b, hg = divmod(c, NCORES // B)
        dsl = slice(hg * DSL, (hg + 1) * DSL)
        in_maps.append({
            "xT": xTb[b],
            "wqT": ca(wq[dsl, :].T.astype(bf)),
            "wkT": ca(wk[dsl, :].T.astype(bf)),
            "wvT": ca(wv[dsl, :].T.astype(bf)),
            "woT": ca(wo[:, dsl].T.astype(bf)),
            "csa": csa, "csb": csb,
            "mask": mask, "onesb": onesb,
        })
    return in_maps


def kernel(hidden_states, wq, wk, wv, wo, trace=False):
    from concourse.bass_utils import run_bass_kernel_spmd

    if "nc" not in _CACHE:
        _CACHE["nc"] = _build_nc()
    nc = _CACHE["nc"]

    in_maps = _host_inputs(
        np.asarray(hidden_states), np.asarray(wq), np.asarray(wk),
        np.asarray(wv), np.asarray(wo),
    )
    res = run_bass_kernel_spmd(nc, in_maps, core_ids=list(range(NCORES)),
                               trace=trace)
    y = np.zeros((B, S, H), np.float32)
    for c in range(NCORES):
        y[c // (NCORES // B)] += res.results[c]["out"]
    if trace:
        return y, res
    return y


# revision 21
# speedup vs baseline: 1.1899x; 1.1899x over previous
"""Self-contained Trainium2 kernel for nn_AMDOptimizedAttention.

Reference computes, for B=2, S=2048, H=2048, nh=16, hd=128:
    q/k/v = hs @ w{q,k,v}.T  (torch Linear convention)
    q, k  = rope(q), rope(k)
    out   = causal_softmax(q @ k.T / sqrt(hd)) @ v
    y     = out @ wo.T

Sharding (Megatron-style tensor parallel over heads + data parallel over
batch): core c handles batch c//4, heads 4*(c%4) .. 4*(c%4)+3.  Each core
computes a partial y for its batch (row-sharded wo); host sums the 4
partials per batch (the "all-reduce" is done on host since kernel() must
return the full output anyway).

v2 layout: everything SBUF-resident per core, no DRAM spill.
  - x (hidden.T), v, exp(scores), wo in bf16; q/k (the softmax-sensitive
    chain) in float32r (fp22).  All matmuls accumulate fp32 in PSUM and
    run at 1 col/cycle.
  - scores computed transposed [k, q]; softmax sum over k via a
    ones-column matmul accumulated in PSUM next to the PV matmul; the
    1/sum row is broadcast across partitions with a K=1 matmul.
  - v-projection runs first, then k, then q, so attention on head h can
    start while later heads still project.
"""

import sys

if "/opt/trn_rl_repo" not in sys.path:
    sys.path.insert(0, "/opt/trn_rl_repo")

import numpy as np

B, S, H = 2, 2048, 2048
NH, HD = 16, 128
P = 128
NCORES = 8
HPC = 4              # heads per core
DSL = HPC * HD       # 512: per-core slice of the hidden dim
KO = H // P          # 16 contraction chunks for projections
TBP = 512            # projection token-block
QB = 512             # attention query-block
NQB = S // QB        # 4
SCALE = 1.0 / np.sqrt(HD)
ROPE_BASE = 10000.0
NEG = -1.0e30

_CACHE = {}


def _build_nc():
    import concourse.mybir as mybir
    from concourse import bacc
    from concourse.tile import TileContext

    f32 = mybir.dt.float32
    f32r = mybir.dt.float32r
    bf16 = mybir.dt.bfloat16
    Alu = mybir.AluOpType
    Act = mybir.ActivationFunctionType

    nc = bacc.Bacc("TRN2", target_bir_lowering=False)

    xT = nc.declare_dram_parameter("xT", [H, S], bf16, isOutput=False)
    wqT = nc.declare_dram_parameter("wqT", [H, DSL], bf16, isOutput=False)
    wkT = nc.declare_dram_parameter("wkT", [H, DSL], bf16, isOutput=False)
    wvT = nc.declare_dram_parameter("wvT", [H, DSL], bf16, isOutput=False)
    woT = nc.declare_dram_parameter("woT", [DSL, H], bf16, isOutput=False)
    # rope tables packed [128, S]: rows 0:64 cos, rows 64:128 sin
    csa = nc.declare_dram_parameter("csa", [P, S], f32, isOutput=False)
    csb = nc.declare_dram_parameter("csb", [P, S], f32, isOutput=False)
    maskp = nc.declare_dram_parameter("mask", [QB // P, P, QB], bf16, isOutput=False)
    onesb = nc.declare_dram_parameter("onesb", [P, P], bf16, isOutput=False)
    yout = nc.declare_dram_parameter("out", [S, H], f32, isOutput=True)

    xTr = xT.rearrange("(ko p) t -> p ko t", p=P)
    wT = {"q": wqT, "k": wkT, "v": wvT}
    wTr = {k: v.rearrange("(ko p) d -> p ko d", p=P) for k, v in wT.items()}

    def mm(ps, lhsT, rhs, start, stop):
        nc.tensor.matmul(ps, lhsT, rhs, start=start, stop=stop)

    with TileContext(nc) as tc, nc.allow_low_precision(
        reason="bf16/f32r staging is deliberate; matmuls accumulate in f32 PSUM"
    ):
        with (
            tc.tile_pool(name="res", bufs=1) as rpool,
            tc.tile_pool(name="xres", bufs=1) as xpool,
            tc.tile_pool(name="wvpool", bufs=1) as wvpool,
        ):
            # ---- residents ----
            xs = [xpool.tile([P, KO, S // 4], bf16, tag=f"xs{g}", name=f"xs{g}")
                  for g in range(4)]            # x.T, token-column chunks
            qT = [rpool.tile([P, S], bf16, tag=f"qT{h}", name=f"qT{h}")
                  for h in range(HPC)]
            kT = [rpool.tile([P, S], bf16, tag=f"kT{h}", name=f"kT{h}")
                  for h in range(HPC)]
            vs = rpool.tile([P, KO, DSL], bf16, tag="vs", name="vs")
            ao = [rpool.tile([P, S], bf16, tag=f"ao{h}", name=f"ao{h}")
                  for h in range(HPC)]
            wv = wvpool.tile([P, KO, DSL], bf16, tag="wv", name="wv")
            nc.sync.dma_start(wv[:], wTr["v"][:])
            TC = S // 4
            for g in range(4):
                nc.sync.dma_start(xs[g][:], xTr[:, :, g * TC:(g + 1) * TC])
            csA = rpool.tile([P, S], f32, tag="csA", name="csA")
            nc.sync.dma_start(csA[:], csa[:])
            csB = rpool.tile([P, S], f32, tag="csB", name="csB")
            nc.sync.dma_start(csB[:], csb[:])
            masks = rpool.tile([P, QB // P, QB], bf16, tag="masks", name="masks")
            nc.sync.dma_start(masks[:], maskp.rearrange("j p f -> p j f"))
            oneb = rpool.tile([P, P], bf16, tag="oneb", name="oneb")
            nc.sync.dma_start(oneb[:], onesb[:])
            wos = rpool.tile([P, DSL // P, H], bf16, tag="wos", name="wos")
            nc.sync.dma_start(wos[:], woT.rearrange("(ko p) e -> p ko e", p=P))

            def xsl(ko, t0, t1):
                g = t0 // TC
                assert t1 <= (g + 1) * TC
                return xs[g][:, ko, t0 - g * TC:t1 - g * TC]

            # ---- projections ----
            with (
                tc.tile_pool(name="wstream", bufs=2) as wpool,
                tc.tile_pool(name="ropetmp", bufs=2) as rtpool,
                tc.tile_pool(name="psA", bufs=1, space="PSUM") as psA,
            ):
                # k projections first (PE can start on 0.26MB of weights),
                # then v, then q
                NTB = S // TBP
                def v_projection():
                    for tt in range(KO):
                        pv = psA.tile([P, DSL], f32, tag="pv", name="pv", bufs=2)
                        for ko in range(KO):
                            mm(pv, xsl(ko, tt * P, (tt + 1) * P), wv[:, ko, :],
                               ko == 0, ko == KO - 1)
                        nc.vector.tensor_copy(vs[:, tt, :], pv[:])

                for pj, dstl in (("k", kT), ("q", qT)):
                    if pj == "q":
                        v_projection()
                    for h in range(HPC):
                        dsl = slice(h * P, (h + 1) * P)
                        wph = wpool.tile([P, KO, P], bf16, tag="wph", name="wph")
                        nc.sync.dma_start(wph[:], wTr[pj][:, :, dsl])
                        pss4 = {}
                        for pair in range(NTB // 2):
                            for i in range(2):
                                tb = pair * 2 + i
                                pss4[tb] = psA.tile([P, TBP], f32, tag=f"psqk{i}",
                                                    name=f"psqk{i}", bufs=2)
                            for ko in range(KO):
                                for tb in (pair * 2, pair * 2 + 1):
                                    mm(pss4[tb], wph[:, ko, :],
                                       xsl(ko, tb * TBP, (tb + 1) * TBP),
                                       ko == 0, ko == KO - 1)
                        for tb in range(NTB):
                            tslc = slice(tb * TBP, (tb + 1) * TBP)
                            ps = pss4[tb]
                            # m1 = [x1*cos; x2*cos] (SBUF), m2 = [x1*sin; x2*sin] (PSUM)
                            m1 = rtpool.tile([P, TBP], f32, tag="m1", name="m1")
                            m2 = psA.tile([P, TBP], f32, tag="m2", name="m2", bufs=2)
                            dst = dstl[h]
                            nc.vector.tensor_tensor(m1[:], ps[:], csA[:, tslc], Alu.mult)
                            nc.vector.tensor_tensor(m2[:], ps[:], csB[:, tslc], Alu.mult)
                            nc.vector.tensor_tensor(
                                dst[0:64, tslc], m1[0:64, :], m2[64:128, :], Alu.subtract)
                            nc.vector.tensor_tensor(
                                dst[64:128, tslc], m1[64:128, :], m2[0:64, :], Alu.add)

            # ---- attention (scores transposed [k, q]) ----
            with (
                tc.tile_pool(name="et", bufs=4) as epool,
                tc.tile_pool(name="nrm", bufs=3) as npool,
                tc.tile_pool(name="pss", bufs=3, space="PSUM") as pss,
                tc.tile_pool(name="pso", bufs=2, space="PSUM") as pso,
                tc.tile_pool(name="psd", bufs=2, space="PSUM") as psd,
            ):
                for h in range(HPC):
                    for qb in range(NQB):
                        qsl = slice(qb * QB, (qb + 1) * QB)
                        nkt = (qb + 1) * (QB // P)
                        po = pso.tile([P, QB], f32, tag="po", name="po")
                        pd = psd.tile([P, QB], f32, tag="pd", name="pd")
                        for kt in range(nkt):
                            pscr = pss.tile([P, QB], f32, tag="pscr", name="pscr")
                            mm(pscr, kT[h][:, kt * P:(kt + 1) * P],
                               qT[h][:, qsl], True, True)
                            j = kt - qb * (QB // P)
                            if j >= 0:
                                nc.vector.tensor_tensor(
                                    pscr[:], pscr[:], masks[:, j, :], Alu.add
                                )
                            et = epool.tile([P, QB], bf16, tag="et", name="et")
                            nc.scalar.activation(et[:], pscr[:], Act.Exp, scale=float(SCALE))
                            mm(po, vs[:, kt, h * P:(h + 1) * P], et[:],
                               kt == 0, kt == nkt - 1)
                            mm(pd, oneb[:], et[:], kt == 0, kt == nkt - 1)
                        rec = npool.tile([P, QB], f32, tag="rec", name="rec")
                        nc.vector.reciprocal(rec[:], pd[:])
                        nc.vector.tensor_tensor(
                            ao[h][:, qsl], po[:], rec[:], Alu.mult
                        )

            # ---- output projection ----
            with (
                tc.tile_pool(name="ystage", bufs=2) as ypool,
                tc.tile_pool(name="psy", bufs=4, space="PSUM") as psy,
            ):
                for tt in range(S // P):
                    tsl = slice(tt * P, (tt + 1) * P)
                    yst = ypool.tile([P, H], f32, tag="yst", name="yst")
                    for ec in range(H // QB):
                        py = psy.tile([P, QB], f32, tag="py", name="py")
                        for dc in range(DSL // P):
                            mm(py, ao[dc][:, tsl],
                               wos[:, dc, ec * QB:(ec + 1) * QB],
                               dc == 0, dc == DSL // P - 1)
                        nc.vector.tensor_copy(yst[:, ec * QB:(ec + 1) * QB], py[:])
                    nc.sync.dma_start(yout[tsl, :], yst[:])

    nc.finalize()
    return nc


def _host_inputs(hidden_states, wq, wk, wv, wo):
    import ml_dtypes

    f32 = np.float32
    bf = ml_dtypes.bfloat16
    ca = np.ascontiguousarray

    inv = 1.0 / (ROPE_BASE ** (np.arange(0, HD, 2, dtype=f32) / HD))
    t = np.arange(S, dtype=f32)
    fr = np.outer(t, inv)                      # [S, 64]
    cosT = np.cos(fr).T.astype(f32)            # [64, S]
    sinT = np.sin(fr).T.astype(f32)
    csa = ca(np.concatenate([cosT, cosT], axis=0))            # [128, S]
    csb = ca(np.concatenate([sinT, sinT], axis=0))

    jj, pp, ff = np.meshgrid(
        np.arange(QB // P), np.arange(P), np.arange(QB), indexing="ij"
    )
    mask = np.where(jj * P + pp > ff, f32(NEG), f32(0.0)).astype(bf)
    onesb = np.ones((P, P), bf)

    xTb = [ca(hidden_states[b].T.astype(bf)) for b in range(B)]

    in_maps = []
    for c in range(NCORES):
        b, hg = divmod(c, NCORES // B)
        dsl = slice(hg * DSL, (hg + 1) * DSL)
        in_maps.append({
            "xT": xTb[b],
            "wqT": ca(wq[dsl, :].T.astype(bf)),
            "wkT": ca(wk[dsl, :].T.astype(bf)),
            "wvT": ca(wv[dsl, :].T.astype(bf)),
            "woT": ca(wo[:, dsl].T.astype(bf)),
            "csa": csa, "csb": csb,
            "mask": mask, "onesb": onesb,
        })
    return in_maps


def kernel(hidden_states, wq, wk, wv, wo, trace=False):
    from concourse.bass_utils import run_bass_kernel_spmd

    if "nc" not in _CACHE:
        _CACHE["nc"] = _build_nc()
    nc = _CACHE["nc"]

    in_maps = _host_inputs(
        np.asarray(hidden_states), np.asarray(wq), np.asarray(wk),
        np.asarray(wv), np.asarray(wo),
    )
    res = run_bass_kernel_spmd(nc, in_maps, core_ids=list(range(NCORES)),
                               trace=trace)
    y = np.zeros((B, S, H), np.float32)
    for c in range(NCORES):
        y[c // (NCORES // B)] += res.results[c]["out"]
    if trace:
        return y, res
    return y
